# revision 10
# baseline (speedup 1.0000x reference)
"""Bidirectional leaky-ESN (B=8,T=2048,D=64,H=1024,O=16) on 8 TRN2 NeuronCores.

Strategy (v2: half-fp8 DoubleRow)
---------------------------------
Chunked-time ESN as v1: the recurrence is a contraction (decay ~0.56/step),
so each of 16 (batch,direction) sequences splits into C=64 chunks of L=32
steps run in parallel from zero state with a WASH=6 washout.  Per core:
128 sequences (full PE width), 38 serial steps.

With s := h/0.9, W' := 0.9 W:  s_k = 0.1 s_{k-1} + tanh(u_proj_k + W' s_{k-1}).

Per step the W'-contraction over H=1024 splits: K-blocks 0-3 run as 2
fp8-e4m3 DoubleRow matmuls (K=256 each, issue ~78ns = 2x bf16 throughput),
blocks 4-7 as 4 bf16 matmuls (~56ns).  Weights carry a x256 scale
(fp8 W' x32, fp8 state copy x8, bf16 W' x256); tanh applies scale=1/256.
Numpy simulation vs an fp64 oracle puts this at rel-err ~1.5e-2 (gate 2e-2);
full-fp8 fails (2.2e-2), full-bf16 is 4e-3 but ~15% slower.

Elementwise ops are fused into half-H tiles: 2 tanh activations
[128,4x128] (PSUM bank each), 2 DVE scalar_tensor_tensor for the leaky
update, 1 tensor_scalar x8 -> fp8 state copy.  The bf16 master state is
stored for the bf16-readout (w_out stays bf16; fp8 readout costs 2.7e-2).
Readout groups of 4 slots run as 8 N=512 matmuls, interleaved with the
recurrence.  Host reassembles fwd+bwd+bias into [B,T,O].
"""

import numpy as np
import ml_dtypes

bf16 = ml_dtypes.bfloat16
f8 = ml_dtypes.float8_e4m3

B, T, D, H, O = 8, 2048, 64, 1024, 16
A = 0.9           # leaky rate
C = 64            # chunks per (batch, direction)
L = T // C        # 32 real-output steps per chunk
WASH = 6          # washout steps
STEPS = L + WASH  # 38
NCORES = 8
NI = H // 128     # 8 partition blocks of H
KAUG = D + 1      # 65: input dim + bias indicator row
WSCALE = 256.0    # pre-activation scale (tanh applies 1/WSCALE)
F8_W = 32.0       # fp8 W' scale
F8_S = WSCALE / F8_W  # fp8 state scale (8): product matches WSCALE

_cached = {}


def _build_program():
    import concourse.bacc as bacc
    import concourse.mybir as mybir
    from concourse.tile import TileContext

    dt = mybir.dt
    nc = bacc.Bacc(trn_type="TRN2", target_bir_lowering=False, debug=False)

    winT_d = nc.dram_tensor("winT", [KAUG, H], dt.bfloat16, kind="ExternalInput").ap()
    vbuf_d = nc.dram_tensor("vbuf", [KAUG, STEPS * 128], dt.bfloat16,
                            kind="ExternalInput").ap()
    wTb_d = nc.dram_tensor("wTb", [128, 4 * NI * 128], dt.bfloat16,
                           kind="ExternalInput").ap()
    wpf_d = nc.dram_tensor("wpf", [128, 2 * NI * 2 * 128], dt.float8e4,
                           kind="ExternalInput").ap()
    woutT_d = nc.dram_tensor("woutT", [128, NI * O], dt.bfloat16,
                             kind="ExternalInput").ap()
    qout_d = nc.dram_tensor("qout", [O, L * 128], dt.float32, kind="ExternalOutput").ap()

    with TileContext(nc) as tc:
        _body(tc, mybir, winT_d, vbuf_d, wTb_d, wpf_d, woutT_d, qout_d)
    nc.compile()
    return nc


def _body(tc, mybir, winT_d, vbuf_d, wTb_d, wpf_d, woutT_d, qout_d):
    dt = mybir.dt
    nc = tc.nc
    Tanh = mybir.ActivationFunctionType.Tanh
    DR = mybir.MatmulPerfMode.DoubleRow
    MUL = mybir.AluOpType.mult
    ADD = mybir.AluOpType.add
    VHEAD = 10 * 128  # vbuf columns needed before compute starts

    with (
        tc.tile_pool(name="const", bufs=1) as constp,
        tc.tile_pool(name="store", bufs=1) as storep,
        tc.tile_pool(name="strans", bufs=2) as stransp,
        tc.tile_pool(name="z", bufs=2) as zp,
        tc.tile_pool(name="s8", bufs=2) as s8p,
        tc.tile_pool(name="stage", bufs=2) as stgp,
        tc.tile_pool(name="pre", bufs=1, space="PSUM") as prep,
    ):
        # ---- prologue DMAs, split across the two HWDGE queues ----
        winT_sb = constp.tile([KAUG, H], dt.bfloat16, tag="winT", name="winT")
        nc.sync.dma_start(winT_sb[:], winT_d[:])
        vbuf_sb = constp.tile([KAUG, STEPS * 128], dt.bfloat16, tag="vbuf", name="vbuf")
        nc.sync.dma_start(vbuf_sb[:, :VHEAD], vbuf_d[:, :VHEAD])
        wpf_sb = constp.tile([128, 2, NI, 2, 128], dt.float8e4, tag="wpf", name="wpf")
        nc.sync.dma_start(
            wpf_sb[:], wpf_d[:].rearrange("p (a i t m) -> p a i t m", a=2, i=NI, t=2))
        wTb_sb = constp.tile([128, 4, NI, 128], dt.bfloat16, tag="wTb", name="wTb")
        nc.sync.dma_start(
            wTb_sb[:], wTb_d[:].rearrange("p (a i m) -> p a i m", a=4, i=NI))
        woutT_sb = constp.tile([128, NI, O], dt.bfloat16, tag="woutT", name="woutT")
        nc.scalar.dma_start(
            woutT_sb[:], woutT_d[:].rearrange("p (i o) -> p i o", i=NI))
        nc.scalar.dma_start(vbuf_sb[:, VHEAD:], vbuf_d[:, VHEAD:])

        store_sb = storep.tile([128, L, NI, 128], dt.bfloat16, tag="st", name="st")
        inv = 1.0 / WSCALE

        def readout_group(g):
            """q for slots 4g..4g+3: 8 matmuls at N=512 + copy + DMA."""
            pr = prep.tile([O, 4, 128], dt.float32, tag=f"pre{4 + g % 4}",
                           name=f"pr{g}")
            for j in range(NI):
                nc.tensor.matmul(pr[:], woutT_sb[:, j],
                                 store_sb[:, 4 * g:4 * g + 4, j, :],
                                 start=(j == 0), stop=(j == NI - 1))
            stg = stgp.tile([O, 4, 128], dt.float32, tag="stg", name=f"stg{g}")
            nc.scalar.copy(stg[:], pr[:])
            nc.sync.dma_start(
                qout_d[:, g * 512:(g + 1) * 512],
                stg[:].rearrange("p a b -> p (a b)"))

        s_prev = None   # [128, NI, 128] bf16 AP of previous state
        s8_prev = None  # [128, 4, 128] f8 AP (blocks 0-3, x8)
        for k in range(STEPS):
            vk = vbuf_sb[:, k * 128:(k + 1) * 128]
            m = k - WASH
            if m >= 0:
                sdst = store_sb[:, m]
            else:
                sdst = stransp.tile([128, NI, 128], dt.bfloat16, tag="str",
                                    name=f"str{k}")[:]
            zt = zp.tile([128, NI, 128], dt.bfloat16, tag="z", name=f"z{k}")
            zdst = sdst if k == 0 else zt[:]
            # one PSUM bank per H-block; u-inj hoisted for blocks 0-3 only
            # (block i's u-inj start waits on ACT_i of the previous step)
            pre = [prep.tile([128, 128], dt.float32, tag=f"pre{i}",
                             name=f"pre{i}_{k}") for i in range(NI)]
            for i in range(4):
                nc.tensor.matmul(pre[i], winT_sb[:, i * 128:(i + 1) * 128], vk,
                                 start=True, stop=(k == 0))
            for i in range(NI):
                if i >= 4:
                    nc.tensor.matmul(pre[i], winT_sb[:, i * 128:(i + 1) * 128],
                                     vk, start=True, stop=(k == 0))
                if k > 0:
                    for jp in range(2):
                        nc.tensor.matmul(pre[i], wpf_sb[:, jp, i],
                                         s8_prev[:, 2 * jp:2 * jp + 2, :],
                                         start=False, stop=False, perf_mode=DR)
                    for j in range(4, NI):
                        nc.tensor.matmul(pre[i], wTb_sb[:, j - 4, i], s_prev[:, j],
                                         start=False, stop=(j == NI - 1))
                nc.scalar.activation(zdst[:, i], pre[i], Tanh, scale=inv)
                if k > 0:
                    # state updates staggered behind their ACTs so next step's
                    # consumers never wait on a late fused op
                    if i == 3:
                        nc.vector.scalar_tensor_tensor(sdst[:, 0:4],
                                                       s_prev[:, 0:4], 0.1,
                                                       zt[:, 0:4], MUL, ADD)
                        s8c = s8p.tile([128, 4, 128], dt.float8e4, tag="s8",
                                       name=f"s8_{k}")
                        nc.vector.tensor_scalar_mul(s8c[:], sdst[:, 0:4], F8_S)
                    elif i == 5 or i == 7:
                        nc.vector.scalar_tensor_tensor(sdst[:, i - 1:i + 1],
                                                       s_prev[:, i - 1:i + 1], 0.1,
                                                       zt[:, i - 1:i + 1], MUL, ADD)
            if k == 0:
                s8c = s8p.tile([128, 4, 128], dt.float8e4, tag="s8", name="s8_0")
                nc.vector.tensor_scalar_mul(s8c[:], sdst[:, 0:4], F8_S)
                # keep the PE busy through k=0's serial ACT/update phase so
                # the HAM clock gate stays warm (results unused)
                for i in range(NI):
                    warm = prep.tile([128, 128], dt.float32, tag=f"pre{i}",
                                     name=f"warm{i}")
                    for r in range(4):
                        nc.tensor.matmul(warm, winT_sb[:, i * 128:(i + 1) * 128],
                                         vk, start=True, stop=True)
            s_prev, s8_prev = sdst, s8c[:]

            # readout as soon as a 4-slot group completes
            if m >= 3 and (m + 1) % 4 == 0:
                readout_group((m + 1) // 4 - 1)


def _prep_inputs(u, w, w_in, w_bias, w_out):
    """Host-side prep: per-core input maps."""
    WT = np.ascontiguousarray((A * w).T).astype(np.float32)  # [j*128+p, i*128+m]
    W4 = WT.reshape(NI, 128, NI, 128)                        # [jblk, p, iblk, m]
    wTb = np.ascontiguousarray(
        (WSCALE * W4[4:]).transpose(1, 0, 2, 3).reshape(128, 4 * NI * 128)
    ).astype(bf16)
    wpf = np.ascontiguousarray(
        (F8_W * W4[:4]).reshape(2, 2, 128, NI, 128)
        .transpose(2, 0, 3, 1, 4).reshape(128, 2 * NI * 2 * 128)
    ).astype(f8)
    winT = np.ascontiguousarray(
        WSCALE * np.concatenate([w_in, w_bias[:, None]], axis=1).T
    ).astype(bf16)                                           # [65, H]
    in_maps = []
    for core in range(NCORES):
        d = core // 4                       # 0 fwd, 1 bwd
        w2 = (A * w_out[1 + d * H:1 + (d + 1) * H, :]).astype(np.float32)  # [H, O]
        woutT = np.ascontiguousarray(
            w2.reshape(NI, 128, O).transpose(1, 0, 2).reshape(128, NI * O)
        ).astype(bf16)
        v = np.zeros((STEPS, KAUG, 128), np.float32)
        ks = np.arange(STEPS)
        for b_loc in range(2):
            b = 2 * (core % 4) + b_loc
            ud = u[b] if d == 0 else u[b, ::-1]
            for c in range(C):
                ts = c * L - WASH + ks
                valid = ts >= 0
                s_idx = b_loc * C + c
                v[valid, :D, s_idx] = ud[ts[valid]]
                v[valid, D, s_idx] = 1.0
        vbuf = np.ascontiguousarray(
            v.transpose(1, 0, 2).reshape(KAUG, STEPS * 128)).astype(bf16)
        in_maps.append({"winT": winT, "vbuf": vbuf, "wTb": wTb, "wpf": wpf,
                        "woutT": woutT})
    return in_maps


def _assemble(results, w_out):
    y = np.zeros((B, T, O), np.float32)
    for core in range(NCORES):
        q = np.asarray(results[core]["qout"], np.float32).reshape(O, L, 128)
        d = core // 4
        for b_loc in range(2):
            b = 2 * (core % 4) + b_loc
            qq = q[:, :, b_loc * C:(b_loc + 1) * C]       # [O, L(m), C(c)]
            tmp = qq.transpose(2, 1, 0).reshape(T, O)     # t = c*L + m
            if d == 0:
                y[b] += tmp
            else:
                y[b, ::-1] += tmp
    y += w_out[0][None, None, :].astype(np.float32)
    return y


def kernel(u, w, w_in, w_bias, w_out):
    from concourse.bass_utils import run_bass_kernel_spmd

    u = np.asarray(u, np.float32)
    w = np.asarray(w, np.float32)
    w_in = np.asarray(w_in, np.float32)
    w_bias = np.asarray(w_bias, np.float32)
    w_out = np.asarray(w_out, np.float32)

    if "nc" not in _cached:
        _cached["nc"] = _build_program()
    nc = _cached["nc"]
    in_maps = _prep_inputs(u, w, w_in, w_bias, w_out)
    res = run_bass_kernel_spmd(nc, in_maps, list(range(NCORES)))
    return _assemble(res.results, w_out)


# revision 12
# speedup vs baseline: 1.2028x; 1.2028x over previous
"""Bidirectional leaky-ESN (B=8,T=2048,D=64,H=1024,O=16) on 8 TRN2 NeuronCores.

Strategy (v2: half-fp8 DoubleRow)
---------------------------------
Chunked-time ESN as v1: the recurrence is a contraction (decay ~0.56/step),
so each of 16 (batch,direction) sequences splits into C=64 chunks of L=32
steps run in parallel from zero state with a WASH=6 washout.  Per core:
128 sequences (full PE width), 38 serial steps.

With s := h/0.9, W' := 0.9 W:  s_k = 0.1 s_{k-1} + tanh(u_proj_k + W' s_{k-1}).

Per step the W'-contraction over H=1024 splits: K-blocks 0-3 run as 2
fp8-e4m3 DoubleRow matmuls (K=256 each, issue ~78ns = 2x bf16 throughput),
blocks 4-7 as 4 bf16 matmuls (~56ns).  Weights carry a x256 scale
(fp8 W' x32, fp8 state copy x8, bf16 W' x256); tanh applies scale=1/256.
Numpy simulation vs an fp64 oracle puts this at rel-err ~1.5e-2 (gate 2e-2);
full-fp8 fails (2.2e-2), full-bf16 is 4e-3 but ~15% slower.

Elementwise ops are fused into half-H tiles: 2 tanh activations
[128,4x128] (PSUM bank each), 2 DVE scalar_tensor_tensor for the leaky
update, 1 tensor_scalar x8 -> fp8 state copy.  The bf16 master state is
stored for the bf16-readout (w_out stays bf16; fp8 readout costs 2.7e-2).
Readout groups of 4 slots run as 8 N=512 matmuls, interleaved with the
recurrence.  Host reassembles fwd+bwd+bias into [B,T,O].
"""

import numpy as np
import ml_dtypes

bf16 = ml_dtypes.bfloat16
f8 = ml_dtypes.float8_e4m3

B, T, D, H, O = 8, 2048, 64, 1024, 16
A = 0.9           # leaky rate
C = 64            # chunks per (batch, direction)
L = T // C        # 32 real-output steps per chunk
WASH = 6          # washout steps
STEPS = L + WASH  # 38
NCORES = 8
NI = H // 128     # 8 partition blocks of H
KAUG = D + 1      # 65: input dim + bias indicator row
WSCALE = 256.0    # pre-activation scale (tanh applies 1/WSCALE)
F8_W = 32.0       # fp8 W' scale
F8_S = WSCALE / F8_W  # fp8 state scale (8): product matches WSCALE

_cached = {}


def _build_program():
    import concourse.bacc as bacc
    import concourse.mybir as mybir
    from concourse.tile import TileContext

    dt = mybir.dt
    nc = bacc.Bacc(trn_type="TRN2", target_bir_lowering=False, debug=False)

    winT_d = nc.dram_tensor("winT", [KAUG, H], dt.bfloat16, kind="ExternalInput").ap()
    vbuf_d = nc.dram_tensor("vbuf", [KAUG, STEPS * 128], dt.bfloat16,
                            kind="ExternalInput").ap()
    wTb_d = nc.dram_tensor("wTb", [128, 4 * NI * 128], dt.bfloat16,
                           kind="ExternalInput").ap()
    wpf_d = nc.dram_tensor("wpf", [128, 2 * NI * 2 * 128], dt.float8e4,
                           kind="ExternalInput").ap()
    woutT_d = nc.dram_tensor("woutT", [128, NI * O], dt.bfloat16,
                             kind="ExternalInput").ap()
    qout_d = nc.dram_tensor("qout", [O, L * 128], dt.float32, kind="ExternalOutput").ap()

    with TileContext(nc) as tc:
        _body(tc, mybir, winT_d, vbuf_d, wTb_d, wpf_d, woutT_d, qout_d)
    nc.compile()
    return nc


def _body(tc, mybir, winT_d, vbuf_d, wTb_d, wpf_d, woutT_d, qout_d):
    dt = mybir.dt
    nc = tc.nc
    Tanh = mybir.ActivationFunctionType.Tanh
    DR = mybir.MatmulPerfMode.DoubleRow
    MUL = mybir.AluOpType.mult
    ADD = mybir.AluOpType.add
    VHEAD = 10 * 128  # vbuf columns needed before compute starts

    with (
        tc.tile_pool(name="const", bufs=1) as constp,
        tc.tile_pool(name="store", bufs=1) as storep,
        tc.tile_pool(name="strans", bufs=2) as stransp,
        tc.tile_pool(name="z", bufs=2) as zp,
        tc.tile_pool(name="s8", bufs=2) as s8p,
        tc.tile_pool(name="stage", bufs=2) as stgp,
        tc.tile_pool(name="pre", bufs=1, space="PSUM") as prep,
    ):
        # ---- prologue DMAs, split across the two HWDGE queues ----
        winT_sb = constp.tile([KAUG, H], dt.bfloat16, tag="winT", name="winT")
        nc.sync.dma_start(winT_sb[:], winT_d[:])
        vbuf_sb = constp.tile([KAUG, STEPS * 128], dt.bfloat16, tag="vbuf", name="vbuf")
        nc.sync.dma_start(vbuf_sb[:, :VHEAD], vbuf_d[:, :VHEAD])
        wpf_sb = constp.tile([128, 2, NI, 2, 128], dt.float8e4, tag="wpf", name="wpf")
        nc.sync.dma_start(
            wpf_sb[:], wpf_d[:].rearrange("p (a i t m) -> p a i t m", a=2, i=NI, t=2))
        wTb_sb = constp.tile([128, 4, NI, 128], dt.bfloat16, tag="wTb", name="wTb")
        nc.scalar.dma_start(
            wTb_sb[:], wTb_d[:].rearrange("p (a i m) -> p a i m", a=4, i=NI))
        woutT_sb = constp.tile([128, NI, O], dt.bfloat16, tag="woutT", name="woutT")
        nc.scalar.dma_start(
            woutT_sb[:], woutT_d[:].rearrange("p (i o) -> p i o", i=NI))
        nc.scalar.dma_start(vbuf_sb[:, VHEAD:], vbuf_d[:, VHEAD:])

        store_sb = storep.tile([128, L, NI, 128], dt.bfloat16, tag="st", name="st")
        inv = 1.0 / WSCALE

        def readout_group(g):
            """q for slots 4g..4g+3: 8 matmuls at N=512 + copy + DMA."""
            pr = prep.tile([O, 4, 128], dt.float32, tag=f"pre{4 + g % 4}",
                           name=f"pr{g}")
            for j in range(NI):
                nc.tensor.matmul(pr[:], woutT_sb[:, j],
                                 store_sb[:, 4 * g:4 * g + 4, j, :],
                                 start=(j == 0), stop=(j == NI - 1))
            stg = stgp.tile([O, 4, 128], dt.float32, tag="stg", name=f"stg{g}")
            nc.scalar.copy(stg[:], pr[:])
            nc.sync.dma_start(
                qout_d[:, g * 512:(g + 1) * 512],
                stg[:].rearrange("p a b -> p (a b)"))

        s_prev = None   # [128, NI, 128] bf16 AP of previous state
        s8_prev = None  # [128, 4, 128] f8 AP (blocks 0-3, x8)
        for k in range(STEPS):
            vk = vbuf_sb[:, k * 128:(k + 1) * 128]
            m = k - WASH
            if m >= 0:
                sdst = store_sb[:, m]
            else:
                sdst = stransp.tile([128, NI, 128], dt.bfloat16, tag="str",
                                    name=f"str{k}")[:]
            zt = zp.tile([128, NI, 128], dt.bfloat16, tag="z", name=f"z{k}")
            zdst = sdst if k == 0 else zt[:]
            # one PSUM bank per H-block; u-inj hoisted for blocks 0-3 only
            # (block i's u-inj start waits on ACT_i of the previous step)
            pre = [prep.tile([128, 128], dt.float32, tag=f"pre{i}",
                             name=f"pre{i}_{k}") for i in range(NI)]
            for i in range(4):
                nc.tensor.matmul(pre[i], winT_sb[:, i * 128:(i + 1) * 128], vk,
                                 start=True, stop=(k == 0))
            for i in range(NI):
                if i >= 4:
                    nc.tensor.matmul(pre[i], winT_sb[:, i * 128:(i + 1) * 128],
                                     vk, start=True, stop=(k == 0))
                if k > 0:
                    for jp in range(2):
                        nc.tensor.matmul(pre[i], wpf_sb[:, jp, i],
                                         s8_prev[:, 2 * jp:2 * jp + 2, :],
                                         start=False, stop=False, perf_mode=DR)
                    for j in range(4, NI):
                        nc.tensor.matmul(pre[i], wTb_sb[:, j - 4, i], s_prev[:, j],
                                         start=False, stop=(j == NI - 1))
                nc.scalar.activation(zdst[:, i], pre[i], Tanh, scale=inv)
                if k > 0:
                    # state updates staggered behind their ACTs so next step's
                    # consumers never wait on a late fused op
                    if i == 3:
                        nc.vector.scalar_tensor_tensor(sdst[:, 0:4],
                                                       s_prev[:, 0:4], 0.1,
                                                       zt[:, 0:4], MUL, ADD)
                        s8c = s8p.tile([128, 4, 128], dt.float8e4, tag="s8",
                                       name=f"s8_{k}")
                        nc.vector.tensor_scalar_mul(s8c[:], sdst[:, 0:4], F8_S)
                    elif i == 5 or i == 7:
                        nc.vector.scalar_tensor_tensor(sdst[:, i - 1:i + 1],
                                                       s_prev[:, i - 1:i + 1], 0.1,
                                                       zt[:, i - 1:i + 1], MUL, ADD)
            if k == 0:
                s8c = s8p.tile([128, 4, 128], dt.float8e4, tag="s8", name="s8_0")
                nc.vector.tensor_scalar_mul(s8c[:], sdst[:, 0:4], F8_S)
            s_prev, s8_prev = sdst, s8c[:]

            # readout as soon as a 4-slot group completes
            if m >= 3 and (m + 1) % 4 == 0:
                readout_group((m + 1) // 4 - 1)


def _prep_inputs(u, w, w_in, w_bias, w_out):
    """Host-side prep: per-core input maps."""
    WT = np.ascontiguousarray((A * w).T).astype(np.float32)  # [j*128+p, i*128+m]
    W4 = WT.reshape(NI, 128, NI, 128)                        # [jblk, p, iblk, m]
    wTb = np.ascontiguousarray(
        (WSCALE * W4[4:]).transpose(1, 0, 2, 3).reshape(128, 4 * NI * 128)
    ).astype(bf16)
    wpf = np.ascontiguousarray(
        (F8_W * W4[:4]).reshape(2, 2, 128, NI, 128)
        .transpose(2, 0, 3, 1, 4).reshape(128, 2 * NI * 2 * 128)
    ).astype(f8)
    winT = np.ascontiguousarray(
        WSCALE * np.concatenate([w_in, w_bias[:, None]], axis=1).T
    ).astype(bf16)                                           # [65, H]
    in_maps = []
    for core in range(NCORES):
        d = core // 4                       # 0 fwd, 1 bwd
        w2 = (A * w_out[1 + d * H:1 + (d + 1) * H, :]).astype(np.float32)  # [H, O]
        woutT = np.ascontiguousarray(
            w2.reshape(NI, 128, O).transpose(1, 0, 2).reshape(128, NI * O)
        ).astype(bf16)
        v = np.zeros((STEPS, KAUG, 128), np.float32)
        ks = np.arange(STEPS)
        for b_loc in range(2):
            b = 2 * (core % 4) + b_loc
            ud = u[b] if d == 0 else u[b, ::-1]
            for c in range(C):
                ts = c * L - WASH + ks
                valid = ts >= 0
                s_idx = b_loc * C + c
                v[valid, :D, s_idx] = ud[ts[valid]]
                v[valid, D, s_idx] = 1.0
        vbuf = np.ascontiguousarray(
            v.transpose(1, 0, 2).reshape(KAUG, STEPS * 128)).astype(bf16)
        in_maps.append({"winT": winT, "vbuf": vbuf, "wTb": wTb, "wpf": wpf,
                        "woutT": woutT})
    return in_maps


def _assemble(results, w_out):
    y = np.zeros((B, T, O), np.float32)
    for core in range(NCORES):
        q = np.asarray(results[core]["qout"], np.float32).reshape(O, L, 128)
        d = core // 4
        for b_loc in range(2):
            b = 2 * (core % 4) + b_loc
            qq = q[:, :, b_loc * C:(b_loc + 1) * C]       # [O, L(m), C(c)]
            tmp = qq.transpose(2, 1, 0).reshape(T, O)     # t = c*L + m
            if d == 0:
                y[b] += tmp
            else:
                y[b, ::-1] += tmp
    y += w_out[0][None, None, :].astype(np.float32)
    return y


def kernel(u, w, w_in, w_bias, w_out):
    from concourse.bass_utils import run_bass_kernel_spmd

    u = np.asarray(u, np.float32)
    w = np.asarray(w, np.float32)
    w_in = np.asarray(w_in, np.float32)
    w_bias = np.asarray(w_bias, np.float32)
    w_out = np.asarray(w_out, np.float32)

    if "nc" not in _cached:
        _cached["nc"] = _build_program()
    nc = _cached["nc"]
    in_maps = _prep_inputs(u, w, w_in, w_bias, w_out)
    res = run_bass_kernel_spmd(nc, in_maps, list(range(NCORES)))
    return _assemble(res.results, w_out)
